# revision 1
# baseline (speedup 1.0000x reference)
"""Block-sparse flash attention on 8 TRN2 NeuronCores.

Problem: q,k,v [16, 8192, 64] fp32, block_mask [128,128] int32 (64x64 blocks).
out[h] = softmax_masked(q[h] @ k[h].T / 8) @ v[h].

Strategy (per core = 2 heads, mask shared across heads):
  - S^T layout: keys on partitions, queries on free dim.  Partitions stacked
    [64 keys of head0 | 64 keys of head1] so both heads' score blocks for key
    block j share one 128-partition tile (mask is head-independent).
  - QK matmul: block-diagonal lhsT = [[K0_j^T, 0], [0, K1_j^T]] (contract dim
    = [d h0 | d h1]), rhs = stacked Q^T, streaming only the query blocks that
    are valid for key block j (packed, host-computed from the mask).
  - exp split across TWO engines (the baseline was 99.6% ScalarE-bound):
    ScalarE does exact exp (scale=1/8 fused); VectorE does a Schraudolph-style
    exp in bf16-bit domain: bf16_bits(e^(s/8)) ~= int16(s*(2^7*log2e/8)
    + (127*2^7 + sigma)), one tensor_scalar (mult,add) with int16 output
    bitcast onto the bf16 P tile.  Regions are assigned greedily to balance
    predicted engine busy time.  sigma tuned so softmax normalization cancels
    most of the ~2% sawtooth error (end-to-end rel err ~1e-2 < 2e-2 budget).
  - PV: lhsT = [V_hj | ones | pad] (ones column yields the softmax denominator
    in output row 64), rhs = the packed exp'd tile; accumulated over key
    blocks into per-(head, chunk-parity) O'^T PSUM tiles at dense query
    positions.  Alternate-head matmuls use disjoint PE row groups and run
    concurrently (~2x PV throughput).
  - CHUNK=512 queries; O' accumulators are one PSUM bank per head, leaving
    room for THREE score-region buffers so the QK stream only waits on the
    exp of region i-3 (two buffers measurably stall the PE behind exp
    latency).  Epilogue copies are split across both engines.
  - Query blocks are reordered (greedy + hill climb on the block-mask
    overlap graph) so valid runs merge within chunks: ~18% fewer matmuls,
    longer streams, less LDWEIGHTS pressure; host unpermutes the output.
  - Host: pre-transposes/stacks Q,K,V into bf16, divides numerator by the
    denominator row and transposes the output back.
No max-subtraction is needed: logits ~ N(0,1), exp stays tiny vs fp32 range.
"""

import os

import numpy as np
import ml_dtypes

H, N, D = 16, 8192, 64
B = 64              # mask block size
NB = N // B         # 128 blocks
NCORES = 8
HPC = H // NCORES   # heads per core
CHUNK = 512         # queries per output chunk (PSUM-resident O' accumulator)
NCHUNK = N // CHUNK
QBC = CHUNK // B    # query blocks per chunk
REGION_W = 1024     # packed score-region width (2 PSUM banks)
BANK_W = 512        # fp32 columns per PSUM bank
VT_W = NB * 65 + 63  # V tile layout width (64 V cols + 1 ones col per block, padded)
BF16 = ml_dtypes.bfloat16

# Schraudolph exp on VectorE: int16(x * EXPA + EXPB) bit-viewed as bf16.
# EXPA folds the 1/8 logit scale: 2^7 * log2(e) / 8.  EXPB = 127*2^7 + sigma.
EXPA = 23.083084261
EXPB = 16256.0 - 8.0

# engine cost model (ns) for greedy balancing, calibrated from traces:
# ACT region = (W+330)/1.2 + sem, DVE region = (W+147)/0.96 + sem
ACT_FIX, ACT_RATE = 325.0, 1 / 1.2
DVE_FIX, DVE_RATE = 230.0, 1 / 0.96

DEBUG = bool(int(os.environ.get("KERNEL_DEBUG", "0")))


def _perm_qbs(mask):
    """Order the 128 query blocks so blocks with similar key-block validity
    sit adjacent within a chunk: valid runs merge, halving-ish the number of
    (fixed-overhead) QK/PV matmuls.  Greedy chunk building + hill climbing on
    the within-chunk adjacent-overlap objective.  Deterministic."""
    O = mask.astype(np.int32) @ mask.astype(np.int32).T  # [qb, qb] overlaps
    used = np.zeros(NB, bool)
    order = []
    for _ in range(NCHUNK):
        cand = np.where(~used)[0]
        start = int(cand[np.argmax(mask[cand].sum(1))])
        used[start] = True
        chunk = [start]
        for _ in range(QBC - 1):
            cand = np.where(~used)[0]
            nxt = int(cand[np.argmax(O[chunk[-1], cand])])
            used[nxt] = True
            chunk.append(nxt)
        order.extend(chunk)
    order = np.array(order)

    ka = np.array([k for k in range(NB - 1) if k % QBC != QBC - 1])

    def edges(o):
        return int(O[o[ka], o[ka + 1]].sum())

    # hill climb: random position swaps + segment reversals
    rng = np.random.default_rng(12345)
    best = edges(order)
    for _ in range(200000):
        mv = rng.integers(0, 2)
        i, j = sorted(rng.integers(0, NB, 2).tolist())
        if i == j:
            continue
        if mv == 0:
            order[i], order[j] = order[j], order[i]
            if (s := edges(order)) >= best:
                best = s
            else:
                order[i], order[j] = order[j], order[i]
        else:
            order[i:j + 1] = order[i:j + 1][::-1]
            if (s := edges(order)) >= best:
                best = s
            else:
                order[i:j + 1] = order[i:j + 1][::-1]
    return order


def _runs(sorted_ints):
    """Maximal runs of consecutive integers -> list of (start, length)."""
    out = []
    for x in sorted_ints:
        if out and x == out[-1][0] + out[-1][1]:
            out[-1][1] += 1
        else:
            out.append([x, 1])
    return [(a, b) for a, b in out]


def _plan(mask):
    """Static schedule from the mask: per chunk, key-block tiles packed into
    score regions.  Returns list over chunks of list of regions; each region is
    (tiles, W) with tiles = [(j, off, runs)], runs = [(local_qb, L)]."""
    plan = []
    for ci in range(NCHUNK):
        items = []
        for j in range(NB):
            qbs = [qb - ci * QBC for qb in range(ci * QBC, (ci + 1) * QBC)
                   if mask[qb, j]]
            if qbs:
                items.append((B * len(qbs), j, _runs(qbs)))
        # First-fit-decreasing: pack key-block tiles into score regions to
        # minimize the number of (fixed-overhead) activation instructions.
        items.sort(key=lambda x: -x[0])
        regions = []  # [ [used_cols, tiles] ]
        for w, j, runs in items:
            for reg in regions:
                if reg[0] + w <= REGION_W:
                    reg[1].append((j, reg[0], runs))
                    reg[0] += w
                    break
            else:
                regions.append([w, [(j, 0, runs)]])
        plan.append([(tiles, used) for used, tiles in regions])
    return plan


def _build_module(mask):
    import concourse.tile as tile
    from concourse import bacc, mybir
    from concourse.alu_op_type import AluOpType

    plan = _plan(mask)

    nc = bacc.Bacc(
        "TRN2",
        debug=False,
        enable_asserts=False,
        target_bir_lowering=False,
        num_devices=NCORES,
    )
    f32 = mybir.dt.float32
    bf16 = mybir.dt.bfloat16
    i16 = mybir.dt.int16
    Exp = mybir.ActivationFunctionType.Exp

    q2t = nc.dram_tensor("q2t", [128, N], bf16, kind="ExternalInput").ap()
    k2t = nc.dram_tensor("k2t", [128, NB * 128], bf16, kind="ExternalInput").ap()
    vt = nc.dram_tensor("vt", [128, VT_W], bf16, kind="ExternalInput").ap()
    out = nc.dram_tensor("out", [HPC, 65, N], f32, kind="ExternalOutput").ap()

    with tile.TileContext(nc) as tc:
        with (
            tc.tile_pool(name="res", bufs=1) as res,
            tc.tile_pool(name="psum", bufs=1, space="PSUM") as psum,
            tc.tile_pool(name="pbuf", bufs=1) as pbuf,
            tc.tile_pool(name="stage", bufs=2) as stage_pool,
        ):
            q2sb = res.tile([128, N], bf16, tag="q2sb", name="q2sb")
            k2sb = res.tile([128, NB * 128], bf16, tag="k2sb", name="k2sb")
            vtsb = res.tile([128, VT_W], bf16, tag="vtsb", name="vtsb")
            # Split resident loads and order them by first use in the static
            # schedule, so the first regions' matmuls start as early as
            # possible instead of waiting for the whole working set.
            use_pos = {}
            pos = 0
            for ci in range(NCHUNK):
                for tiles, _w in plan[ci]:
                    for (j, _off, _r) in tiles:
                        use_pos.setdefault(j, pos)
                        pos += 1
            KG = 4   # k2 piece = 4 key blocks
            kg_order = sorted(range(NB // KG),
                              key=lambda g: min(use_pos.get(j, 1 << 30)
                                                for j in range(g * KG, (g + 1) * KG)))
            VG = 8   # vt piece = 8 key blocks
            vg_order = sorted(range(NB // VG),
                              key=lambda g: min(use_pos.get(j, 1 << 30)
                                                for j in range(g * VG, (g + 1) * VG)))
            nc.sync.dma_start(out=q2sb[:, 0:CHUNK], in_=q2t[:, 0:CHUNK])
            for i in range(max(len(kg_order), len(vg_order) + 1)):
                if i < len(kg_order):
                    g = kg_order[i]
                    lo, hi = g * KG * 128, (g + 1) * KG * 128
                    nc.sync.dma_start(out=k2sb[:, lo:hi], in_=k2t[:, lo:hi])
                if 0 < i <= len(vg_order):
                    g = vg_order[i - 1]
                    lo = g * VG * 65
                    hi = VT_W if g == NB // VG - 1 else (g + 1) * VG * 65
                    nc.sync.dma_start(out=vtsb[:, lo:hi], in_=vt[:, lo:hi])
            for p in range(1, NCHUNK):
                nc.sync.dma_start(
                    out=q2sb[:, p * CHUNK:(p + 1) * CHUNK],
                    in_=q2t[:, p * CHUNK:(p + 1) * CHUNK],
                )

            # O' accumulators: one PSUM bank per head (CHUNK=512).  The
            # lag-3 PV pipeline gives the epilogue copy ~3 regions of cover
            # before the next chunk's first PV reuses the bank.
            o_ps = [
                psum.tile([128, CHUNK], f32, tag=f"o{h}", name=f"o{h}")
                for h in range(2)
            ]
            # THREE score buffers: QK of region i+3 only waits on the exp of
            # region i, decoupling the PE stream from exp-engine latency.
            s_ps = [
                psum.tile([128, REGION_W], f32, tag=f"s{i}", name=f"s{i}")
                for i in range(3)
            ]
            p_sb = [
                pbuf.tile([128, REGION_W], bf16, tag=f"p{i}", name=f"p{i}")
                for i in range(8)
            ]

            # greedy two-engine balance state (ns of queued work per engine)
            load = {"act": 0.0, "dve": 0.0}
            nreg_eng = {"act": 0, "dve": 0}

            def emit_qk(ci, tiles, rb):
                S = s_ps[rb]
                for (j, off, runs) in tiles:
                    lhs = k2sb[:, j * 128:(j + 1) * 128]
                    local = off
                    for (q0, L) in runs:
                        seg = B * L
                        s = 0
                        while s < seg:
                            e = min(seg, ((local + s) // BANK_W + 1) * BANK_W - local)
                            rq = ci * CHUNK + q0 * B + s
                            nc.tensor.matmul(
                                S[:, local + s:local + e],
                                lhsT=lhs,
                                rhs=q2sb[:, rq:rq + (e - s)],
                                start=True, stop=True,
                                skip_group_check=True,
                            )
                            s = e
                        local += seg

            def emit_act(rb, pb, W):
                # Split every region across BOTH engines concurrently: halves
                # the region's exp latency, which is what the QK stream's
                # score-ring WAR actually waits on (engines have headroom).
                hw = (W // 2 + 63) & ~63
                load["act"] += ACT_FIX + hw * ACT_RATE
                nreg_eng["act"] += 1
                nc.scalar.activation(
                    p_sb[pb][:, 0:hw], s_ps[rb][:, 0:hw], Exp, scale=0.125
                )
                if hw < W:
                    load["dve"] += DVE_FIX + (W - hw) * DVE_RATE
                    nreg_eng["dve"] += 1
                    nc.vector.tensor_scalar(
                        p_sb[pb][:, hw:W].bitcast(i16),
                        s_ps[rb][:, hw:W],
                        EXPA, EXPB,
                        op0=AluOpType.mult, op1=AluOpType.add,
                    )

            # The first PV matmul touching each O' PSUM bank per chunk uses
            # start=True: it clears the whole bank's has_written bits and
            # overwrites its own columns; every later matmul (start=False)
            # overwrites where the bit is clear and accumulates where set.
            ft = {"ci": None, "seen": set()}

            def emit_pv(ci, tiles, pb, is_last_of_chunk):
                if ft["ci"] != ci:
                    ft["ci"] = ci
                    ft["seen"] = set()
                P = p_sb[pb]
                for ti, (j, off, runs) in enumerate(tiles):
                    local = off
                    for ri, (q0, L) in enumerate(runs):
                        seg = B * L
                        c0 = q0 * B
                        last = (
                            is_last_of_chunk
                            and ti == len(tiles) - 1
                            and ri == len(runs) - 1
                        )
                        # Alternate heads per piece: disjoint PE row groups
                        # run concurrently.
                        for h in (0, 1):
                            first = h not in ft["seen"]
                            if first:
                                ft["seen"].add(h)
                            nc.tensor.matmul(
                                o_ps[h][:, c0:c0 + seg],
                                lhsT=vtsb[h * 64:(h + 1) * 64,
                                          j * 65:j * 65 + 128],
                                rhs=P[h * 64:(h + 1) * 64,
                                      local:local + seg],
                                start=first, stop=last,
                                skip_group_check=True,
                            )
                        local += seg

            def emit_epilogue(ci):
                # Evacuation: split each head's copy across BOTH engines so
                # the PSUM bank frees in half the latency.
                half = CHUNK // 2
                for h in (0, 1):
                    st = stage_pool.tile(
                        [65, CHUNK], f32, tag=f"st{h}", name=f"st{h}_{ci}"
                    )
                    load["act"] += ACT_FIX + half * ACT_RATE
                    load["dve"] += DVE_FIX + half * DVE_RATE
                    nc.scalar.copy(st[:, 0:half], o_ps[h][0:65, 0:half])
                    nc.vector.tensor_copy(
                        st[:, half:CHUNK], o_ps[h][0:65, half:CHUNK]
                    )
                    nc.sync.dma_start(
                        out=out[h, :, ci * CHUNK:(ci + 1) * CHUNK], in_=st[:]
                    )

            flat = []
            for ci in range(NCHUNK):
                nreg = len(plan[ci])
                for ri, (tiles, W) in enumerate(plan[ci]):
                    flat.append((ci, tiles, W, ri == 0, ri == nreg - 1))

            # Warm the PE HAM clock gate during the initial DMA wait with
            # garbage matmuls on the first q2 piece (the earliest-landing DMA;
            # outputs are overwritten by the real schedule).  24 matmuls span
            # the k2/vt DMA window so the HAM SHORT window flips to 2.4 GHz
            # before the first real QK (a 13us cold dip was measured with 8).
            for w in range(24):
                nc.tensor.matmul(
                    s_ps[1][:, (w % 2) * BANK_W:(w % 2 + 1) * BANK_W],
                    lhsT=q2sb[:, 0:128],
                    rhs=q2sb[:, 0:BANK_W],
                    start=True, stop=True, skip_group_check=True,
                )

            # Software-pipeline with the PV pass lagging THREE regions behind
            # QK/exp, so each region's activation has ~2 QK-regions of cover
            # before its PV consumers run (the 6-deep P2 ring allows this;
            # the score-region WAR distance is unchanged).
            pending = []

            def flush_one():
                pci, ptiles, ppb, pfirst, plast = pending.pop(0)
                emit_pv(pci, ptiles, ppb, plast)
                if plast:
                    emit_epilogue(pci)

            for gi, (ci, tiles, W, first, last) in enumerate(flat):
                rb = gi % 3
                pb = gi % 8
                emit_qk(ci, tiles, rb)
                emit_act(rb, pb, W)
                pending.append((ci, tiles, pb, first, last))
                if len(pending) > 4:
                    flush_one()
            while pending:
                flush_one()

    nc.finalize()

    if DEBUG:
        tot_w = sum(W for regs in plan for (_, W) in regs)
        nregs = sum(len(regs) for regs in plan)
        print(f"[build] regions={nregs} total packed cols={tot_w} "
              f"act_regions={nreg_eng['act']} dve_regions={nreg_eng['dve']} "
              f"pred ACT={load['act'] / 1e3:.1f}us DVE={load['dve'] / 1e3:.1f}us")
    return nc


def _prep_core(qf, kf, vf, h0, h1):
    q2t = np.empty((128, N), BF16)
    q2t[:64] = qf[h0].T
    q2t[64:] = qf[h1].T

    k2t = np.zeros((128, NB * 128), BF16)
    k2t[:64].reshape(64, NB, 128)[:, :, :64] = (
        kf[h0].T.astype(BF16).reshape(64, NB, 64)
    )
    k2t[64:].reshape(64, NB, 128)[:, :, 64:] = (
        kf[h1].T.astype(BF16).reshape(64, NB, 64)
    )

    vt = np.zeros((128, VT_W), BF16)
    vt[:64, :NB * 65].reshape(64, NB, 65)[:, :, :64] = (
        vf[h0].reshape(NB, 64, 64).transpose(1, 0, 2).astype(BF16)
    )
    vt[:64, :NB * 65].reshape(64, NB, 65)[:, :, 64] = 1.0
    vt[64:, :NB * 65].reshape(64, NB, 65)[:, :, :64] = (
        vf[h1].reshape(NB, 64, 64).transpose(1, 0, 2).astype(BF16)
    )
    vt[64:, :NB * 65].reshape(64, NB, 65)[:, :, 64] = 1.0
    return {"q2t": q2t, "k2t": k2t, "vt": vt}


def kernel(q, k, v, block_mask):
    from concourse.bass_utils import run_bass_kernel_spmd

    qf = np.asarray(q, dtype=np.float32)
    kf = np.asarray(k, dtype=np.float32)
    vf = np.asarray(v, dtype=np.float32)
    mask = np.asarray(block_mask) != 0

    perm = _perm_qbs(mask)                      # query-block order on device
    qidx = (perm[:, None] * B + np.arange(B)[None, :]).reshape(-1)

    nc = _build_module(mask[perm])
    qp = qf[:, qidx, :]
    in_maps = [_prep_core(qp, kf, vf, 2 * c, 2 * c + 1) for c in range(NCORES)]

    res = run_bass_kernel_spmd(nc, in_maps, core_ids=list(range(NCORES)))

    o_full = np.empty((H, N, D), dtype=np.float32)
    empty_rows = np.repeat(mask.sum(axis=1) == 0, B)
    for c in range(NCORES):
        ot = res.results[c]["out"]  # [2, 65, N] (queries in permuted order)
        with np.errstate(divide="ignore", invalid="ignore"):
            o = ot[:, :64, :] / ot[:, 64:65, :]
        o_full[2 * c:2 * c + 2, qidx] = o.transpose(0, 2, 1)
    if empty_rows.any():
        o_full[:, empty_rows, :] = np.nan
    return o_full



# revision 24
# speedup vs baseline: 1.2110x; 1.2110x over previous
"""Block-sparse flash attention on 8 TRN2 NeuronCores.

Problem: q,k,v [16, 8192, 64] fp32, block_mask [128,128] int32 (64x64 blocks).
out[h] = softmax_masked(q[h] @ k[h].T / 8) @ v[h].

Strategy (per core = 2 heads, mask shared across heads):
  - S^T layout: keys on partitions, queries on free dim.  Partitions stacked
    [64 keys of head0 | 64 keys of head1] so both heads' score blocks for key
    block j share one 128-partition tile (mask is head-independent).
  - QK matmul: block-diagonal lhsT = [[K0_j^T, 0], [0, K1_j^T]] (contract dim
    = [d h0 | d h1]), rhs = stacked Q^T, streaming only the query blocks that
    are valid for key block j (packed, host-computed from the mask).
  - exp split across TWO engines (the baseline was 99.6% ScalarE-bound):
    ScalarE does exact exp (scale=1/8 fused); VectorE does a Schraudolph-style
    exp in bf16-bit domain: bf16_bits(e^(s/8)) ~= int16(s*(2^7*log2e/8)
    + (127*2^7 + sigma)), one tensor_scalar (mult,add) with int16 output
    bitcast onto the bf16 P tile.  Regions are assigned greedily to balance
    predicted engine busy time.  sigma tuned so softmax normalization cancels
    most of the ~2% sawtooth error (end-to-end rel err ~1e-2 < 2e-2 budget).
  - PV: lhsT = [V_hj | ones | pad] (ones column yields the softmax denominator
    in output row 64), rhs = the packed exp'd tile; accumulated over key
    blocks into per-(head, chunk-parity) O'^T PSUM tiles at dense query
    positions.  Alternate-head matmuls use disjoint PE row groups and run
    concurrently (~2x PV throughput).
  - CHUNK=512 queries; O' accumulators are one PSUM bank per head, leaving
    room for THREE score-region buffers so the QK stream only waits on the
    exp of region i-3 (two buffers measurably stall the PE behind exp
    latency).  Epilogue copies are split across both engines.
  - Query blocks are reordered (greedy + hill climb on the block-mask
    overlap graph) so valid runs merge within chunks: ~18% fewer matmuls,
    longer streams, less LDWEIGHTS pressure; host unpermutes the output.
  - Host: pre-transposes/stacks Q,K,V into bf16, divides numerator by the
    denominator row and transposes the output back.
No max-subtraction is needed: logits ~ N(0,1), exp stays tiny vs fp32 range.
"""

import os

import numpy as np
import ml_dtypes

H, N, D = 16, 8192, 64
B = 64              # mask block size
NB = N // B         # 128 blocks
NCORES = 8
HPC = H // NCORES   # heads per core
CHUNK = 512         # queries per output chunk (PSUM-resident O' accumulator)
NCHUNK = N // CHUNK
QBC = CHUNK // B    # query blocks per chunk
REGION_W = 1024     # packed score-region width (2 PSUM banks)
BANK_W = 512        # fp32 columns per PSUM bank
VT_W = NB * 65 + 63  # V tile layout width (64 V cols + 1 ones col per block, padded)
BF16 = ml_dtypes.bfloat16

# Schraudolph exp on VectorE: int16(x * EXPA + EXPB) bit-viewed as bf16.
# EXPA folds the 1/8 logit scale: 2^7 * log2(e) / 8.  EXPB = 127*2^7 + sigma.
EXPA = 23.083084261
EXPB = 16256.0 - 8.0

# engine cost model (ns) for greedy balancing, calibrated from traces:
# ACT region = (W+330)/1.2 + sem, DVE region = (W+147)/0.96 + sem
ACT_FIX, ACT_RATE = 325.0, 1 / 1.2
DVE_FIX, DVE_RATE = 230.0, 1 / 0.96

DEBUG = bool(int(os.environ.get("KERNEL_DEBUG", "0")))


def _perm_qbs(mask):
    """Order the 128 query blocks so blocks with similar key-block validity
    sit adjacent within a chunk: valid runs merge, halving-ish the number of
    (fixed-overhead) QK/PV matmuls.  Greedy chunk building + hill climbing on
    the within-chunk adjacent-overlap objective.  Deterministic."""
    O = mask.astype(np.int32) @ mask.astype(np.int32).T  # [qb, qb] overlaps
    used = np.zeros(NB, bool)
    order = []
    for _ in range(NCHUNK):
        cand = np.where(~used)[0]
        start = int(cand[np.argmax(mask[cand].sum(1))])
        used[start] = True
        chunk = [start]
        for _ in range(QBC - 1):
            cand = np.where(~used)[0]
            nxt = int(cand[np.argmax(O[chunk[-1], cand])])
            used[nxt] = True
            chunk.append(nxt)
        order.extend(chunk)
    order = np.array(order)

    ka = np.array([k for k in range(NB - 1) if k % QBC != QBC - 1])

    def edges(o):
        return int(O[o[ka], o[ka + 1]].sum())

    # hill climb: random position swaps + segment reversals
    rng = np.random.default_rng(12345)
    best = edges(order)
    for _ in range(200000):
        mv = rng.integers(0, 2)
        i, j = sorted(rng.integers(0, NB, 2).tolist())
        if i == j:
            continue
        if mv == 0:
            order[i], order[j] = order[j], order[i]
            if (s := edges(order)) >= best:
                best = s
            else:
                order[i], order[j] = order[j], order[i]
        else:
            order[i:j + 1] = order[i:j + 1][::-1]
            if (s := edges(order)) >= best:
                best = s
            else:
                order[i:j + 1] = order[i:j + 1][::-1]
    return order


def _runs(sorted_ints):
    """Maximal runs of consecutive integers -> list of (start, length)."""
    out = []
    for x in sorted_ints:
        if out and x == out[-1][0] + out[-1][1]:
            out[-1][1] += 1
        else:
            out.append([x, 1])
    return [(a, b) for a, b in out]


def _plan(mask):
    """Static schedule from the mask: per chunk, key-block tiles packed into
    score regions.  Returns list over chunks of list of regions; each region is
    (tiles, W) with tiles = [(j, off, runs)], runs = [(local_qb, L)]."""
    plan = []
    for ci in range(NCHUNK):
        items = []
        for j in range(NB):
            qbs = [qb - ci * QBC for qb in range(ci * QBC, (ci + 1) * QBC)
                   if mask[qb, j]]
            if qbs:
                items.append((B * len(qbs), j, _runs(qbs)))
        # First-fit-decreasing: pack key-block tiles into score regions to
        # minimize the number of (fixed-overhead) activation instructions.
        items.sort(key=lambda x: -x[0])
        regions = []  # [ [used_cols, tiles] ]
        for w, j, runs in items:
            for reg in regions:
                if reg[0] + w <= REGION_W:
                    reg[1].append((j, reg[0], runs))
                    reg[0] += w
                    break
            else:
                regions.append([w, [(j, 0, runs)]])
        plan.append([(tiles, used) for used, tiles in regions])
    return plan


def _build_module(mask):
    import concourse.tile as tile
    from concourse import bacc, mybir
    from concourse.alu_op_type import AluOpType

    plan = _plan(mask)

    nc = bacc.Bacc(
        "TRN2",
        debug=False,
        enable_asserts=False,
        target_bir_lowering=False,
        num_devices=NCORES,
    )
    f32 = mybir.dt.float32
    bf16 = mybir.dt.bfloat16
    i16 = mybir.dt.int16
    Exp = mybir.ActivationFunctionType.Exp

    q2t = nc.dram_tensor("q2t", [128, N], bf16, kind="ExternalInput").ap()
    k2t = nc.dram_tensor("k2t", [128, NB * 128], bf16, kind="ExternalInput").ap()
    vt = nc.dram_tensor("vt", [128, VT_W], bf16, kind="ExternalInput").ap()
    out = nc.dram_tensor("out", [HPC, 65, N], f32, kind="ExternalOutput").ap()

    with tile.TileContext(nc) as tc:
        with (
            tc.tile_pool(name="res", bufs=1) as res,
            tc.tile_pool(name="psum", bufs=1, space="PSUM") as psum,
            tc.tile_pool(name="pbuf", bufs=1) as pbuf,
            tc.tile_pool(name="stage", bufs=2) as stage_pool,
        ):
            q2sb = res.tile([128, N], bf16, tag="q2sb", name="q2sb")
            k2sb = res.tile([128, NB * 128], bf16, tag="k2sb", name="k2sb")
            vtsb = res.tile([128, VT_W], bf16, tag="vtsb", name="vtsb")
            # Split resident loads and order them by first use in the static
            # schedule, so the first regions' matmuls start as early as
            # possible instead of waiting for the whole working set.
            use_pos = {}
            pos = 0
            for ci in range(NCHUNK):
                for tiles, _w in plan[ci]:
                    for (j, _off, _r) in tiles:
                        use_pos.setdefault(j, pos)
                        pos += 1
            KG = 4   # k2 piece = 4 key blocks
            kg_order = sorted(range(NB // KG),
                              key=lambda g: min(use_pos.get(j, 1 << 30)
                                                for j in range(g * KG, (g + 1) * KG)))
            VG = 8   # vt piece = 8 key blocks
            vg_order = sorted(range(NB // VG),
                              key=lambda g: min(use_pos.get(j, 1 << 30)
                                                for j in range(g * VG, (g + 1) * VG)))
            nc.sync.dma_start(out=q2sb[:, 0:CHUNK], in_=q2t[:, 0:CHUNK])
            for i in range(max(len(kg_order), len(vg_order) + 1)):
                if i < len(kg_order):
                    g = kg_order[i]
                    lo, hi = g * KG * 128, (g + 1) * KG * 128
                    nc.sync.dma_start(out=k2sb[:, lo:hi], in_=k2t[:, lo:hi])
                if 0 < i <= len(vg_order):
                    g = vg_order[i - 1]
                    lo = g * VG * 65
                    hi = VT_W if g == NB // VG - 1 else (g + 1) * VG * 65
                    nc.sync.dma_start(out=vtsb[:, lo:hi], in_=vt[:, lo:hi])
            for p in range(1, NCHUNK):
                nc.sync.dma_start(
                    out=q2sb[:, p * CHUNK:(p + 1) * CHUNK],
                    in_=q2t[:, p * CHUNK:(p + 1) * CHUNK],
                )

            # O' accumulators: one PSUM bank per head (CHUNK=512).  The
            # lag-3 PV pipeline gives the epilogue copy ~3 regions of cover
            # before the next chunk's first PV reuses the bank.
            o_ps = [
                psum.tile([128, CHUNK], f32, tag=f"o{h}", name=f"o{h}")
                for h in range(2)
            ]
            # THREE score buffers: QK of region i+3 only waits on the exp of
            # region i, decoupling the PE stream from exp-engine latency.
            s_ps = [
                psum.tile([128, REGION_W], f32, tag=f"s{i}", name=f"s{i}")
                for i in range(3)
            ]
            p_sb = [
                pbuf.tile([128, REGION_W], bf16, tag=f"p{i}", name=f"p{i}")
                for i in range(8)
            ]

            # greedy two-engine balance state (ns of queued work per engine)
            load = {"act": 0.0, "dve": 0.0}
            nreg_eng = {"act": 0, "dve": 0}

            def emit_qk(ci, tiles, rb):
                S = s_ps[rb]
                for (j, off, runs) in tiles:
                    lhs = k2sb[:, j * 128:(j + 1) * 128]
                    local = off
                    for (q0, L) in runs:
                        seg = B * L
                        s = 0
                        while s < seg:
                            e = min(seg, ((local + s) // BANK_W + 1) * BANK_W - local)
                            rq = ci * CHUNK + q0 * B + s
                            nc.tensor.matmul(
                                S[:, local + s:local + e],
                                lhsT=lhs,
                                rhs=q2sb[:, rq:rq + (e - s)],
                                start=True, stop=True,
                                skip_group_check=True,
                            )
                            s = e
                        local += seg

            def emit_act(rb, pb, W):
                # Split every region across BOTH engines concurrently: halves
                # the region's exp latency, which is what the QK stream's
                # score-ring WAR actually waits on (engines have headroom).
                hw = (W // 2 + 63) & ~63
                load["act"] += ACT_FIX + hw * ACT_RATE
                nreg_eng["act"] += 1
                nc.scalar.activation(
                    p_sb[pb][:, 0:hw], s_ps[rb][:, 0:hw], Exp, scale=0.125
                )
                if hw < W:
                    load["dve"] += DVE_FIX + (W - hw) * DVE_RATE
                    nreg_eng["dve"] += 1
                    nc.vector.tensor_scalar(
                        p_sb[pb][:, hw:W].bitcast(i16),
                        s_ps[rb][:, hw:W],
                        EXPA, EXPB,
                        op0=AluOpType.mult, op1=AluOpType.add,
                    )

            # The first PV matmul touching each O' PSUM bank per chunk uses
            # start=True: it clears the whole bank's has_written bits and
            # overwrites its own columns; every later matmul (start=False)
            # overwrites where the bit is clear and accumulates where set.
            ft = {"ci": None, "seen": set()}

            def emit_pv(ci, tiles, pb, is_last_of_chunk):
                if ft["ci"] != ci:
                    ft["ci"] = ci
                    ft["seen"] = set()
                P = p_sb[pb]
                for ti, (j, off, runs) in enumerate(tiles):
                    local = off
                    for ri, (q0, L) in enumerate(runs):
                        seg = B * L
                        c0 = q0 * B
                        last = (
                            is_last_of_chunk
                            and ti == len(tiles) - 1
                            and ri == len(runs) - 1
                        )
                        # Alternate heads per piece: disjoint PE row groups
                        # run concurrently.
                        for h in (0, 1):
                            first = h not in ft["seen"]
                            if first:
                                ft["seen"].add(h)
                            nc.tensor.matmul(
                                o_ps[h][:, c0:c0 + seg],
                                lhsT=vtsb[h * 64:(h + 1) * 64,
                                          j * 65:j * 65 + 128],
                                rhs=P[h * 64:(h + 1) * 64,
                                      local:local + seg],
                                start=first, stop=last,
                                skip_group_check=True,
                            )
                        local += seg

            def emit_epilogue(ci):
                # Evacuation: split each head's copy across BOTH engines so
                # the PSUM bank frees in half the latency.
                half = CHUNK // 2
                for h in (0, 1):
                    st = stage_pool.tile(
                        [65, CHUNK], f32, tag=f"st{h}", name=f"st{h}_{ci}"
                    )
                    load["act"] += ACT_FIX + half * ACT_RATE
                    load["dve"] += DVE_FIX + half * DVE_RATE
                    nc.scalar.copy(st[:, 0:half], o_ps[h][0:65, 0:half])
                    nc.vector.tensor_copy(
                        st[:, half:CHUNK], o_ps[h][0:65, half:CHUNK]
                    )
                    nc.sync.dma_start(
                        out=out[h, :, ci * CHUNK:(ci + 1) * CHUNK], in_=st[:]
                    )

            flat = []
            for ci in range(NCHUNK):
                nreg = len(plan[ci])
                for ri, (tiles, W) in enumerate(plan[ci]):
                    flat.append((ci, tiles, W, ri == 0, ri == nreg - 1))

            # Warm the PE HAM clock gate during the initial DMA wait with
            # garbage matmuls on the first q2 piece (the earliest-landing DMA;
            # outputs are overwritten by the real schedule).  24 matmuls span
            # the k2/vt DMA window so the HAM SHORT window flips to 2.4 GHz
            # before the first real QK (a 13us cold dip was measured with 8).
            for w in range(24):
                nc.tensor.matmul(
                    s_ps[1][:, (w % 2) * BANK_W:(w % 2 + 1) * BANK_W],
                    lhsT=q2sb[:, 0:128],
                    rhs=q2sb[:, 0:BANK_W],
                    start=True, stop=True, skip_group_check=True,
                )

            # Software-pipeline with the PV pass lagging THREE regions behind
            # QK/exp, so each region's activation has ~2 QK-regions of cover
            # before its PV consumers run (the 6-deep P2 ring allows this;
            # the score-region WAR distance is unchanged).
            pending = []

            def flush_one():
                pci, ptiles, ppb, pfirst, plast = pending.pop(0)
                emit_pv(pci, ptiles, ppb, plast)
                if plast:
                    emit_epilogue(pci)

            for gi, (ci, tiles, W, first, last) in enumerate(flat):
                rb = gi % 3
                pb = gi % 8
                emit_qk(ci, tiles, rb)
                emit_act(rb, pb, W)
                pending.append((ci, tiles, pb, first, last))
                if len(pending) > 4:
                    flush_one()
            while pending:
                flush_one()

    nc.finalize()

    if DEBUG:
        tot_w = sum(W for regs in plan for (_, W) in regs)
        nregs = sum(len(regs) for regs in plan)
        print(f"[build] regions={nregs} total packed cols={tot_w} "
              f"act_regions={nreg_eng['act']} dve_regions={nreg_eng['dve']} "
              f"pred ACT={load['act'] / 1e3:.1f}us DVE={load['dve'] / 1e3:.1f}us")
    return nc


def _prep_core(qf, kf, vf, h0, h1):
    q2t = np.empty((128, N), BF16)
    q2t[:64] = qf[h0].T
    q2t[64:] = qf[h1].T

    k2t = np.zeros((128, NB * 128), BF16)
    k2t[:64].reshape(64, NB, 128)[:, :, :64] = (
        kf[h0].T.astype(BF16).reshape(64, NB, 64)
    )
    k2t[64:].reshape(64, NB, 128)[:, :, 64:] = (
        kf[h1].T.astype(BF16).reshape(64, NB, 64)
    )

    vt = np.zeros((128, VT_W), BF16)
    vt[:64, :NB * 65].reshape(64, NB, 65)[:, :, :64] = (
        vf[h0].reshape(NB, 64, 64).transpose(1, 0, 2).astype(BF16)
    )
    vt[:64, :NB * 65].reshape(64, NB, 65)[:, :, 64] = 1.0
    vt[64:, :NB * 65].reshape(64, NB, 65)[:, :, :64] = (
        vf[h1].reshape(NB, 64, 64).transpose(1, 0, 2).astype(BF16)
    )
    vt[64:, :NB * 65].reshape(64, NB, 65)[:, :, 64] = 1.0
    return {"q2t": q2t, "k2t": k2t, "vt": vt}


def kernel(q, k, v, block_mask):
    from concourse.bass_utils import run_bass_kernel_spmd

    qf = np.asarray(q, dtype=np.float32)
    kf = np.asarray(k, dtype=np.float32)
    vf = np.asarray(v, dtype=np.float32)
    mask = np.asarray(block_mask) != 0

    perm = _perm_qbs(mask)                      # query-block order on device
    qidx = (perm[:, None] * B + np.arange(B)[None, :]).reshape(-1)

    nc = _build_module(mask[perm])
    qp = qf[:, qidx, :]
    in_maps = [_prep_core(qp, kf, vf, 2 * c, 2 * c + 1) for c in range(NCORES)]

    res = run_bass_kernel_spmd(nc, in_maps, core_ids=list(range(NCORES)))

    o_full = np.empty((H, N, D), dtype=np.float32)
    empty_rows = np.repeat(mask.sum(axis=1) == 0, B)
    for c in range(NCORES):
        ot = res.results[c]["out"]  # [2, 65, N] (queries in permuted order)
        with np.errstate(divide="ignore", invalid="ignore"):
            o = ot[:, :64, :] / ot[:, 64:65, :]
        o_full[2 * c:2 * c + 2, qidx] = o.transpose(0, 2, 1)
    if empty_rows.any():
        o_full[:, empty_rows, :] = np.nan
    return o_full

